# revision 23
# baseline (speedup 1.0000x reference)
"""Trainium2 Bass kernel for nn_Attention (dense transformer block:
QKV proj + RoPE + causal GQA attention + o_proj), SPMD over 8 NeuronCores.

Sharding: core c -> (batch b = c//4, head-group g = c%4). Each core computes
4 query heads + its kv head for one batch, then the head outputs are
AllGather'd within the 4-core batch group and each core computes a disjoint
512-column slice of the o_proj output.

v2 vs baseline:
 - x arrives pre-transposed from the host (xT [E,S]) -> no on-chip x
   transposes (was 256 PE transposes + 64 scalar copies per core).
 - everything except PSUM and the final output is bf16: halves DMA +
   collective traffic, doubles DVE elementwise throughput.
 - softmax denominator accumulated on DVE (bf16 adds) instead of 160
   PE ones-matmuls; only the final partition-reduce runs on PE.
 - exp: off-diagonal score tiles are exp'd in [128,1024] pairs (halves
   ACTIVATE instruction overhead); diagonal tiles are exp'd only on the
   causal-valid column range.
 - causal mask: one [128,128] triangular bf16 multiply per diagonal tile
   (invalid prefix memset to 0 on gpsimd) instead of full [128,512] fp32
   multiplies.
 - softmax normalization (reduce/recip/broadcast/scale) for chunk qc is
   emitted during chunk qc+1 so the PE never stalls on the DVE chain.
 - RoPE rotate-half multiplies run on gpsimd, cos-mult + add on DVE.
"""

import sys
import time

sys.path.insert(0, "/opt/trn_rl_repo")

import numpy as np

import concourse.bass as bass
import concourse.mybir as mybir
import concourse.tile as tile
from concourse import bacc
from concourse.masks import make_identity

F32 = mybir.dt.float32
BF16 = mybir.dt.bfloat16
P = 128
HD = 128            # head dim
NHL = 4             # query heads per core
E = 2048            # hidden
DQ = NHL * HD       # 512, local q-projection width / o-slice width
SCALE = 1.0 / np.sqrt(np.float32(HD))
REPLICA_GROUPS = [[0, 1, 2, 3], [4, 5, 6, 7]]
NO_COLLECTIVE = False  # replace AllGather with a local DMA (timeline-sim only)
AG_HALVES = 1          # AllGathers per head (1, 2, or 4; must divide NQC)
DEN_DVE = True         # softmax denominator via DVE bf16 adds (else PE matmuls)
ROPE_GPSIMD = False    # rotate-half multiplies on gpsimd (else DVE)
DEN_RED = "pe"         # denominator partition-reduce: "pe" | "gpsimd"
LAG = 6                # pv(kt-LAG) emitted after scores(kt): hides exp+mask


def build_program(S=2048, reps=1, n_cores=8):
    """Build the per-core SPMD Bass program. Returns compiled nc."""
    ST = S // P          # 128-row tiles along sequence
    NQC = S // 512       # 512-wide chunks along sequence
    ET = E // P          # 16 tiles along hidden

    nc = bacc.Bacc("TRN2", target_bir_lowering=False, debug=False,
                   num_devices=n_cores)

    xT_in = nc.declare_dram_parameter("xT", [E, S], BF16, isOutput=False)
    wqT_in = nc.declare_dram_parameter("wqT", [E, DQ], BF16, isOutput=False)
    wkT_in = nc.declare_dram_parameter("wkT", [E, HD], BF16, isOutput=False)
    wvT_in = nc.declare_dram_parameter("wvT", [E, HD], BF16, isOutput=False)
    woT_in = nc.declare_dram_parameter("woT", [E, DQ], BF16, isOutput=False)
    cosT_in = nc.declare_dram_parameter("cosT", [HD, S], BF16, isOutput=False)
    sinT_in = nc.declare_dram_parameter("sinT", [HD, S], BF16, isOutput=False)
    out_d = nc.declare_dram_parameter("out", [DQ, S], F32, isOutput=True)

    with tile.TileContext(nc) as tc:
        with nc.allow_low_precision(reason="bf16 operands; tolerance 2e-2"):
            _emit(tc, nc, S, ST, NQC, ET, reps,
                  xT_in, wqT_in, wkT_in, wvT_in, woT_in, cosT_in, sinT_in,
                  out_d)

    nc.compile()
    return nc


def _emit(tc, nc, S, ST, NQC, ET, reps,
          xT_in, wqT_in, wkT_in, wvT_in, woT_in, cosT_in, sinT_in, out_d):
    from contextlib import ExitStack

    ctx = ExitStack()
    with ctx:
        const = ctx.enter_context(tc.tile_pool(name="const", bufs=1))
        qkv = ctx.enter_context(tc.tile_pool(name="qkv", bufs=2))
        dram = ctx.enter_context(tc.tile_pool(name="dram", bufs=2, space="DRAM"))

        # ---- constants ----
        ident = const.tile([P, P], BF16)
        make_identity(nc, ident[:])
        # triangular causal mask for the 128x128 diagonal blocks:
        # tri[k, j] = 1 if j >= k else 0
        tri = const.tile([P, P], BF16)
        nc.gpsimd.memset(tri[:], 1.0)
        nc.gpsimd.affine_select(
            out=tri[:], in_=tri[:],
            compare_op=mybir.AluOpType.is_ge,
            fill=0.0, base=0, pattern=[[1, P]],
            channel_multiplier=-1,
        )
        zeros_t = const.tile([P, 512], BF16)
        nc.gpsimd.memset(zeros_t[:], 0.0)
        ones_stage = const.tile([P, P], BF16)
        nc.gpsimd.memset(ones_stage[:], 1.0)
        ones_red = const.tile([P, 1], BF16)
        nc.vector.tensor_copy(ones_red[:], ones_stage[:, 0:1])
        ones_col = const.tile([1, P], BF16)
        nc.vector.tensor_copy(ones_col[:], ones_stage[0:1, :])


        pools = {
            "xn": ctx.enter_context(tc.tile_pool(name="xn", bufs=2)),
            "trig": ctx.enter_context(tc.tile_pool(name="trig", bufs=1)),
            "rope": ctx.enter_context(tc.tile_pool(name="rope", bufs=3)),
            "vt": ctx.enter_context(tc.tile_pool(name="vt", bufs=2)),
            "ex": ctx.enter_context(tc.tile_pool(name="ex", bufs=5)),
            "dn": ctx.enter_context(tc.tile_pool(name="dn", bufs=2)),
            "bc": ctx.enter_context(tc.tile_pool(name="bc", bufs=2)),
            "oh": ctx.enter_context(tc.tile_pool(name="oh", bufs=2)),
            "af": ctx.enter_context(tc.tile_pool(name="af", bufs=2)),
            "of": ctx.enter_context(tc.tile_pool(name="of", bufs=2)),
            # PSUM budget (8 banks): pj 2 + pt 1 + sc (2x[P,1024]) 4 + pv 1.
            # o_proj po tiles share the sc ring (doses run between chunks).
            "pj_ps": ctx.enter_context(tc.tile_pool(name="pj_ps", bufs=2, space="PSUM")),
            "pt_ps": ctx.enter_context(tc.tile_pool(name="pt_ps", bufs=1, space="PSUM")),
            "sc_ps": ctx.enter_context(tc.tile_pool(name="sc_ps", bufs=2, space="PSUM")),
            "pv_ps": ctx.enter_context(tc.tile_pool(name="pv_ps", bufs=1, space="PSUM")),
        }
        pools["qkv"] = qkv
        pools["dram"] = dram
        for rep in range(reps):
            _emit_rep(tc, nc, S, ST, NQC, ET, rep, pools,
                      xT_in, wqT_in, wkT_in, wvT_in, woT_in, cosT_in, sinT_in,
                      out_d, ident, tri, ones_red, ones_col, zeros_t)


def _emit_rep(tc, nc, S, ST, NQC, ET, rep, pools,
              xT_in, wqT_in, wkT_in, wvT_in, woT_in, cosT_in, sinT_in,
              out_d, ident, tri, ones_red, ones_col, zeros_t):
    import concourse.bass_isa as bass_isa

    NHALF = min(AG_HALVES, NQC)
    cph = NQC // NHALF
    out_r = out_d.rearrange("(ot p) s -> p ot s", p=P)

    # per-rep Q/K/V buffers (double-buffered pool) so rep r+1's projections
    # can overlap rep r's attention tail
    QT_sb = pools["qkv"].tile([P, NHL, S], BF16, name="QT", tag="QT")
    KT_sb = pools["qkv"].tile([P, S], BF16, name="KT", tag="KT")
    V_sb = pools["qkv"].tile([P, ST, HD], BF16, name="V", tag="V")
    SH = S // NHALF
    agin = [[pools["dram"].tile([P, SH], BF16, name=f"agin{h}_{hf}",
                                tag=f"agin{h}_{hf}")
             for hf in range(NHALF)] for h in range(NHL)]
    agout = [[pools["dram"].tile([4 * P, SH], BF16, name=f"agout{h}_{hf}",
                                 tag=f"agout{h}_{hf}")
              for hf in range(NHALF)] for h in range(NHL)]

    if True:
        xn_pool = pools["xn"]
        trig_pool = pools["trig"]
        rope_pool = pools["rope"]
        vt_pool = pools["vt"]
        ex_pool = pools["ex"]
        dn_pool = pools["dn"]
        bc_pool = pools["bc"]
        oh_pool = pools["oh"]
        af_pool = pools["af"]
        of_pool = pools["of"]
        pj_ps = pools["pj_ps"]
        pt_ps = pools["pt_ps"]
        sc_ps = pools["sc_ps"]
        pv_ps = pools["pv_ps"]

        cosT_sb = trig_pool.tile([P, S], BF16)
        sinT_sb = trig_pool.tile([P, S], BF16)
        wqT_sb = trig_pool.tile([P, ET, DQ], BF16)
        wkT_sb = trig_pool.tile([P, ET, HD], BF16)
        wvT_sb = trig_pool.tile([P, ET, HD], BF16)
        woT_sb = trig_pool.tile([P, ET, DQ], BF16)

        x_r = xT_in.rearrange("(et p) s -> p et s", p=P)
        wq_r = wqT_in.rearrange("(et p) d -> p et d", p=P)
        wk_r = wkT_in.rearrange("(et p) d -> p et d", p=P)
        wv_r = wvT_in.rearrange("(et p) d -> p et d", p=P)
        wo_r = woT_in.rearrange("(et p) d -> p et d", p=P)

        # ---------------- per-chunk emitters ----------------

        def emit_proj(qc):
            s0 = qc * 512
            xc = xn_pool.tile([P, ET, 512], BF16, name="xc", tag="xc")
            for q4 in range(4):
                nc.sync.dma_start(xc[:, q4 * 4:(q4 + 1) * 4, :],
                                  x_r[:, q4 * 4:(q4 + 1) * 4, s0:s0 + 512])
                if qc == 0:
                    # interleave weight-slice DMAs with the first chunk's
                    # x loads so the first matmuls aren't starved
                    et4 = slice(q4 * 4, (q4 + 1) * 4)
                    nc.sync.dma_start(wqT_sb[:, et4, :], wq_r[:, et4, :])
                    nc.sync.dma_start(wkT_sb[:, et4, :], wk_r[:, et4, :])
                    nc.sync.dma_start(wvT_sb[:, et4, :], wv_r[:, et4, :])
            # cos/sin stream in per-chunk; wo isn't needed until the first
            # o_proj dose, so its load is deferred off the critical start
            nc.sync.dma_start(cosT_sb[:, s0:s0 + 512], cosT_in[:, s0:s0 + 512])
            nc.sync.dma_start(sinT_sb[:, s0:s0 + 512], sinT_in[:, s0:s0 + 512])
            if qc == min(1, NQC - 1):
                for q4 in range(4):
                    et4 = slice(q4 * 4, (q4 + 1) * 4)
                    nc.sync.dma_start(woT_sb[:, et4, :], wo_r[:, et4, :])

            cos_c = cosT_sb[:, s0:s0 + 512]
            sin_c = sinT_sb[:, s0:s0 + 512]
            for d6 in range(6):
                pp = pj_ps.tile([P, 512], F32, name="pp", tag="pp")
                for et in range(ET):
                    if d6 < 4:
                        lhsT = wqT_sb[:, et, d6 * HD:(d6 + 1) * HD]
                    elif d6 == 4:
                        lhsT = wkT_sb[:, et, :]
                    else:
                        lhsT = wvT_sb[:, et, :]
                    nc.tensor.matmul(pp[:], lhsT, xc[:, et, :],
                                     start=(et == 0), stop=(et == ET - 1))
                if d6 < 5:
                    dst = (QT_sb[:, d6, s0:s0 + 512] if d6 < 4
                           else KT_sb[:, s0:s0 + 512])
                    ppb = rope_pool.tile([P, 512], BF16, name="ppb", tag="ppb")
                    ppr = rope_pool.tile([P, 512], BF16, name="ppr", tag="ppr")
                    nc.scalar.copy(ppb[:], pp[:])
                    # rotate-half folded into the PSUM->SBUF evacuation
                    nc.scalar.copy(ppr[0:64, :], pp[64:128, :])
                    nc.scalar.copy(ppr[64:128, :], pp[0:64, :])
                    t1 = rope_pool.tile([P, 512], BF16, name="t1", tag="t1")
                    t2 = rope_pool.tile([P, 512], BF16, name="t2", tag="t2")
                    nc.vector.tensor_tensor(t1[:], ppb[:], cos_c,
                                            mybir.AluOpType.mult)
                    # sinT arrives with rows 0:64 pre-negated (host side)
                    eng = nc.gpsimd if ROPE_GPSIMD else nc.vector
                    eng.tensor_tensor(t2[:], ppr[:], sin_c,
                                      mybir.AluOpType.mult)
                    nc.vector.tensor_tensor(dst, t1[:], t2[:],
                                            mybir.AluOpType.add)
                else:
                    vts = vt_pool.tile([P, 512], BF16, name="vts", tag="vts")
                    nc.scalar.copy(vts[:], pp[:])
                    for st4 in range(4):
                        pv_t = pt_ps.tile([P, P], BF16, name="pvt",
                                          tag="ptile")
                        nc.tensor.transpose(pv_t[:],
                                            vts[:, st4 * P:(st4 + 1) * P],
                                            ident[:])
                        nc.scalar.copy(V_sb[:, qc * 4 + st4, :], pv_t[:])

        pending = [None]

        def norm_stage1(pv, denacc, h, qc):
            if DEN_RED == "pe":
                # partition-reduce the denominator on PE (half a sc-ring tile)
                pdred = sc_ps.tile([P, 1024], F32, name="pdred", tag="ps2")
                nc.tensor.matmul(pdred[0:1, 0:512], ones_red[:], denacc[:],
                                 start=True, stop=True)
                rec = bc_pool.tile([1, 512], BF16, name="rec", tag="rec")
                nc.vector.reciprocal(rec[:], pdred[0:1, 0:512])
                return (pv, rec, h, qc)
            else:
                denred = dn_pool.tile([P, 512], BF16, name="denred",
                                      tag="denred")
                nc.gpsimd.partition_all_reduce(
                    denred[:], denacc[:], channels=P,
                    reduce_op=bass_isa.ReduceOp.add)
                bcr = bc_pool.tile([P, 512], BF16, name="bcr", tag="bcr")
                nc.vector.reciprocal(bcr[:], denred[:])
                return (pv, bcr, h, qc)

        def norm_stage2(st):
            pv, rec, h, qc = st
            if DEN_RED == "pe":
                pbc = sc_ps.tile([P, 1024], F32, name="pbc", tag="ps2")
                nc.tensor.matmul(pbc[:, 0:512], ones_col[:], rec[:],
                                 start=True, stop=True)
                bcr = bc_pool.tile([P, 512], BF16, name="bcr", tag="bcr")
                nc.scalar.copy(bcr[:], pbc[:, 0:512])
            else:
                bcr = rec
            outH = oh_pool.tile([P, 512], BF16, name="outH", tag="outH")
            nc.vector.tensor_tensor(outH[:], pv[:], bcr[:],
                                    mybir.AluOpType.mult)
            hf = qc // cph
            qh0 = (qc - hf * cph) * 512
            nc.sync.dma_start(agin[h][hf][:, qh0:qh0 + 512], outH[:])

        def flush_pending():
            if pending[0] is not None:
                norm_stage2(norm_stage1(*pending[0]))
                pending[0] = None

        def emit_attn(h, qc):
            q0 = qc * 512
            nkt = 4 * qc + 4
            noff = 4 * qc            # fully-valid (off-diagonal) tiles
            qT = QT_sb[:, h, q0:q0 + 512]
            pv = pv_ps.tile([P, 512], F32, name="pv", tag="pv")
            denacc = dn_pool.tile([P, 512], BF16, name="denacc",
                                  tag="denacc")
            exs = [None] * nkt
            stage_state = [None]

            def emit_pv(j, last):
                t = j - noff
                c0 = P * t if t > 0 else 0   # diag tiles: ex is 0 below c0
                nc.tensor.matmul(pv[:, c0:512], V_sb[:, j, :],
                                 exs[j][:, c0:512],
                                 start=(j == 0), stop=last)

            ps2 = None
            ex2 = None
            for kt in range(nkt):
                half = kt % 2
                off = half * 512
                if half == 0:
                    ps2 = sc_ps.tile([P, 1024], F32, name="ps2", tag="ps2")
                    ex2 = ex_pool.tile([P, 1024], BF16, name="ex2",
                                       tag="ex2")
                t = kt - noff        # >= 0 -> diagonal tile
                if t >= 0:
                    c0 = P * t
                    nc.tensor.matmul(ps2[:, off + c0:off + 512],
                                     KT_sb[:, kt * P:(kt + 1) * P],
                                     qT[:, c0:512],
                                     start=True, stop=True)
                    # per-tile exp on the causal-valid range only
                    nc.scalar.activation(
                        ex2[:, off + c0:off + 512],
                        ps2[:, off + c0:off + 512],
                        mybir.ActivationFunctionType.Exp,
                        scale=float(SCALE))
                    if c0 > 0:
                        nc.vector.tensor_copy(ex2[:, off:off + c0],
                                              zeros_t[:, 0:c0])
                    nc.vector.tensor_tensor(
                        ex2[:, off + c0:off + c0 + P],
                        ex2[:, off + c0:off + c0 + P],
                        tri[:], mybir.AluOpType.mult)
                else:
                    nc.tensor.matmul(ps2[:, off:off + 512],
                                     KT_sb[:, kt * P:(kt + 1) * P],
                                     qT, start=True, stop=True)
                    if half == 1:
                        # paired exp over both halves
                        nc.scalar.activation(
                            ex2[:, :], ps2[:, :],
                            mybir.ActivationFunctionType.Exp,
                            scale=float(SCALE))
                exs[kt] = ex2[:, off:off + 512]
                # denominator accumulation (DVE bf16). For paired off-diag
                # tiles BOTH adds must come after the pair's exp emission
                # (at the odd kt), else the even tile's add binds to a stale
                # writer of the recycled ex2 buffer.
                den_tiles = ()
                if t >= 0:
                    den_tiles = (kt,)
                elif half == 1:
                    den_tiles = (kt - 1, kt)
                for dk in den_tiles:
                    if dk == 0:
                        nc.vector.tensor_copy(denacc[:], exs[0])
                    else:
                        nc.vector.tensor_add(denacc[:], denacc[:], exs[dk])
                if kt == 1 and pending[0] is not None:
                    stage_state[0] = norm_stage1(*pending[0])
                    pending[0] = None
                if kt == 3 and stage_state[0] is not None:
                    norm_stage2(stage_state[0])
                    stage_state[0] = None
                if kt >= LAG:
                    emit_pv(kt - LAG, last=False)
            for j in range(max(0, nkt - LAG), nkt):
                emit_pv(j, last=(j == nkt - 1))
            if stage_state[0] is not None:
                norm_stage2(stage_state[0])
                stage_state[0] = None

            # softmax normalization is deferred: stages are emitted during
            # the NEXT chunk so the PE pipeline never waits on the chain
            pending[0] = (pv, denacc, h, qc)

        def emit_ag(hf):
            for h in range(NHL):
                if NO_COLLECTIVE:
                    for mt in range(4):
                        nc.sync.dma_start(
                            agout[h][hf][mt * P:(mt + 1) * P, :],
                            agin[h][hf][:])
                else:
                    nc.gpsimd.collective_compute(
                        "AllGather", mybir.AluOpType.bypass,
                        replica_groups=REPLICA_GROUPS,
                        ins=[agin[h][hf].opt()],
                        outs=[agout[h][hf].opt()])

        def prefetch_af(hf, sch):
            afs = []
            for h in range(NHL):
                af = af_pool.tile([P, 4, 512], BF16, name=f"af{h}",
                                  tag=f"af{h}")
                ag_r = agout[h][hf].rearrange("(mt p) s -> p mt s", p=P)
                nc.sync.dma_start(
                    af[:], ag_r[:, :, sch * 512:(sch + 1) * 512])
                afs.append(af)
            return afs

        def emit_oproj_ot(hf, sch, afs, ot):
            # one o_proj output tile: accumulate over (mt, head) in PSUM --
            # all four heads' AllGathers for this half are done
            sc = hf * cph + sch
            s0 = sc * 512
            po = sc_ps.tile([P, 1024], F32, name="po", tag="ps2")
            first = True
            for mt in range(4):
                for h in range(NHL):
                    nc.tensor.matmul(
                        po[:, 0:512],
                        woT_sb[:, 4 * mt + h, ot * P:(ot + 1) * P],
                        afs[h][:, mt, :],
                        start=first, stop=(mt == 3 and h == NHL - 1))
                    first = False
            of = of_pool.tile([P, 512], F32, name="of", tag="of")
            nc.vector.tensor_copy(of[:], po[:, 0:512])
            nc.sync.dma_start(out_r[:, ot, s0:s0 + 512], of[:])

        # ---------------- qc-major schedule ----------------
        from collections import deque
        dose_q = deque()     # (hf, sch, afs, ot) pending o_proj tiles

        def enqueue_doses(hf):
            for sch in range(cph):
                afs = prefetch_af(hf, sch)
                for ot in range(4):
                    dose_q.append((hf, sch, afs, ot))

        for qc in range(NQC):
            emit_proj(qc)
            for h in range(NHL):
                emit_attn(h, qc)
                if dose_q:
                    emit_oproj_ot(*dose_q.popleft())
            if (qc + 1) % cph == 0:
                hf = qc // cph
                flush_pending()
                emit_ag(hf)
                enqueue_doses(hf)
        flush_pending()
        while dose_q:
            emit_oproj_ot(*dose_q.popleft())


# ======================= host side =======================

_CACHE = {}


def _get_program(S=2048, reps=1):
    key = (S, reps, AG_HALVES, NO_COLLECTIVE, DEN_DVE, ROPE_GPSIMD)
    if key not in _CACHE:
        _CACHE[key] = build_program(S=S, reps=reps)
    return _CACHE[key]


def make_in_maps(x, cos, sin, wq, wk, wv, wo):
    bf = mybir.dt.np(BF16)
    in_maps = []
    cosT = np.ascontiguousarray(cos.T.astype(np.float32)).astype(bf)
    sinT = sin.T.astype(np.float32).copy()
    sinT[:HD // 2, :] *= -1.0   # fold rotate_half sign into the table
    sinT = np.ascontiguousarray(sinT).astype(bf)
    xTs = [np.ascontiguousarray(np.asarray(x[b]).T.astype(np.float32)).astype(bf)
           for b in range(x.shape[0])]
    wqTs = [np.ascontiguousarray(wq[g * DQ:(g + 1) * DQ, :].T.astype(np.float32)).astype(bf)
            for g in range(4)]
    wkTs = [np.ascontiguousarray(wk[g * HD:(g + 1) * HD, :].T.astype(np.float32)).astype(bf)
            for g in range(4)]
    wvTs = [np.ascontiguousarray(wv[g * HD:(g + 1) * HD, :].T.astype(np.float32)).astype(bf)
            for g in range(4)]
    woTs = [np.ascontiguousarray(wo[:, :].T[:, g * DQ:(g + 1) * DQ].astype(np.float32)).astype(bf)
            for g in range(4)]
    for c in range(8):
        b, g = c // 4, c % 4
        in_maps.append({
            "xT": xTs[b],
            "wqT": wqTs[g],
            "wkT": wkTs[g],
            "wvT": wvTs[g],
            "woT": woTs[g],
            "cosT": cosT,
            "sinT": sinT,
        })
    return in_maps


def assemble_output(results, B, S):
    out = np.empty((B, S, E), np.float32)
    for c in range(8):
        b, g = c // 4, c % 4
        out[b][:, g * DQ:(g + 1) * DQ] = results[c]["out"].T
    return out


# ---- inline SPMD runner (PJRT/axon), device-resident inputs ----

class SpmdRunner:
    def __init__(self, nc, n_cores):
        import jax
        from jax.sharding import Mesh, PartitionSpec
        from jax.experimental.shard_map import shard_map
        from concourse import bass2jax
        from concourse.bass2jax import _bass_exec_p, install_neuronx_cc_hook

        install_neuronx_cc_hook()
        self.jax = jax
        self.nc = nc
        self.n_cores = n_cores
        partition_name = (nc.partition_id_tensor.name
                          if nc.partition_id_tensor else None)
        in_names, out_names, out_avals = [], [], []
        zero_outs = []
        for alloc in nc.m.functions[0].allocations:
            if not isinstance(alloc, mybir.MemoryLocationSet):
                continue
            name = alloc.memorylocations[0].name
            if alloc.kind == "ExternalInput":
                if name != partition_name:
                    in_names.append(name)
            elif alloc.kind == "ExternalOutput":
                out_names.append(name)
                shape = tuple(alloc.tensor_shape)
                dtype = mybir.dt.np(alloc.dtype)
                out_avals.append(jax.core.ShapedArray(shape, dtype))
                zero_outs.append(np.zeros(shape, dtype))
        self.in_names, self.out_names = in_names, out_names
        self.out_avals, self.zero_outs = out_avals, zero_outs
        self.n_params = len(in_names)

        all_in = list(in_names) + list(out_names)
        if partition_name is not None:
            all_in.append(partition_name)

        def _body(*args):
            operands = list(args)
            if partition_name is not None:
                operands.append(bass2jax.partition_id_tensor())
            outs = _bass_exec_p.bind(
                *operands, out_avals=tuple(out_avals),
                in_names=tuple(all_in), out_names=tuple(out_names),
                lowering_input_output_aliases=(),
                sim_require_finite=True, sim_require_nnan=True, nc=nc)
            return tuple(outs)

        devices = jax.devices()[:n_cores]
        self.mesh = Mesh(np.asarray(devices), ("core",))
        n_outs = len(out_names)
        in_specs = (PartitionSpec("core"),) * (self.n_params + n_outs)
        out_specs = (PartitionSpec("core"),) * n_outs
        self.fn = jax.jit(
            shard_map(_body, mesh=self.mesh, in_specs=in_specs,
                      out_specs=out_specs, check_rep=False),
            keep_unused=True)
        self.dev_args = None

    def stage_inputs(self, in_maps):
        import jax
        from jax.sharding import PartitionSpec
        per_core = [[np.asarray(m[n]) for n in self.in_names] for m in in_maps]
        concat_in = [
            np.concatenate([per_core[c][i] for c in range(self.n_cores)], axis=0)
            for i in range(self.n_params)]
        concat_zeros = [
            np.zeros((self.n_cores * z.shape[0], *z.shape[1:]), z.dtype)
            for z in self.zero_outs]
        sharding = jax.sharding.NamedSharding(self.mesh, PartitionSpec("core"))
        self.dev_args = [jax.device_put(a, sharding)
                         for a in (*concat_in, *concat_zeros)]
        for a in self.dev_args:
            a.block_until_ready()

    def run(self):
        out_arrs = [np.asarray(o) for o in self.fn(*self.dev_args)]
        return [
            {n: out_arrs[i].reshape(self.n_cores, *self.out_avals[i].shape)[c]
             for i, n in enumerate(self.out_names)}
            for c in range(self.n_cores)]

    def time_exec(self, iters=30, warmup=3):
        import jax
        for _ in range(warmup):
            res = self.fn(*self.dev_args)
        jax.block_until_ready(res)
        t0 = time.perf_counter()
        for _ in range(iters):
            res = self.fn(*self.dev_args)
        jax.block_until_ready(res)
        t1 = time.perf_counter()
        return (t1 - t0) / iters * 1e9


_RUNNER_CACHE = {}


def get_runner(S=2048, reps=1):
    key = (S, reps, AG_HALVES, NO_COLLECTIVE, DEN_DVE, ROPE_GPSIMD)
    if key not in _RUNNER_CACHE:
        nc = _get_program(S=S, reps=reps)
        _RUNNER_CACHE[key] = SpmdRunner(nc, 8)
    return _RUNNER_CACHE[key]


def kernel(x, cos, sin, wq, wk, wv, wo):
    B, S, _ = x.shape
    runner = get_runner(S=S, reps=1)
    runner.stage_inputs(make_in_maps(x, cos, sin, wq, wk, wv, wo))
    results = runner.run()
    return assemble_output(results, B, S)


if __name__ == "__main__":
    # tiny self-test against a local numpy reference
    S = int(sys.argv[1]) if len(sys.argv) > 1 else 512
    rng = np.random.default_rng(0)
    B, H, HKV = 2, 16, 4
    x = rng.standard_normal((B, S, E), dtype=np.float32)
    cos = rng.random((S, HD), dtype=np.float32)
    sin = rng.random((S, HD), dtype=np.float32)
    sc = 0.02
    wq = (rng.standard_normal((H * HD, E), dtype=np.float32) * sc)
    wk = (rng.standard_normal((HKV * HD, E), dtype=np.float32) * sc)
    wv = (rng.standard_normal((HKV * HD, E), dtype=np.float32) * sc)
    wo = (rng.standard_normal((E, H * HD), dtype=np.float32) * sc)

    def ref(x, cos, sin, wq, wk, wv, wo):
        x64 = x.astype(np.float64)
        q = (x64 @ wq.T.astype(np.float64)).reshape(B, S, H, HD)
        k = (x64 @ wk.T.astype(np.float64)).reshape(B, S, HKV, HD)
        v = (x64 @ wv.T.astype(np.float64)).reshape(B, S, HKV, HD)

        def rot(t):
            return np.concatenate([-t[..., HD // 2:], t[..., :HD // 2]], -1)

        c = cos[:, None, :].astype(np.float64)
        s = sin[:, None, :].astype(np.float64)
        q = q * c + rot(q) * s
        k = k * c + rot(k) * s
        k = np.repeat(k, H // HKV, axis=2).transpose(0, 2, 1, 3)
        v = np.repeat(v, H // HKV, axis=2).transpose(0, 2, 1, 3)
        q = q.transpose(0, 2, 1, 3)
        scores = np.einsum("bhqd,bhkd->bhqk", q, k) / np.sqrt(HD)
        mask = np.tril(np.ones((S, S), bool))
        scores = np.where(mask, scores, -np.inf)
        scores -= scores.max(-1, keepdims=True)
        p = np.exp(scores)
        p /= p.sum(-1, keepdims=True)
        o = np.einsum("bhqk,bhkd->bhqd", p, v)
        o = o.transpose(0, 2, 1, 3).reshape(B, S, H * HD)
        return o @ wo.T.astype(np.float64)

    want = ref(x, cos, sin, wq, wk, wv, wo)
    got = kernel(x, cos, sin, wq, wk, wv, wo)
    err = np.abs(got - want).max() / np.abs(want).max()
    print(f"S={S}: rel err (absmax-relative) = {err:.3e}")


# revision 24
# speedup vs baseline: 1.2151x; 1.2151x over previous
"""Trainium2 Bass kernel for nn_Attention (dense transformer block:
QKV proj + RoPE + causal GQA attention + o_proj), SPMD over 8 NeuronCores.

Sharding: core c -> (batch b = c//4, head-group g = c%4). Each core computes
4 query heads + its kv head for one batch, then the head outputs are
AllGather'd within the 4-core batch group and each core computes a disjoint
512-column slice of the o_proj output.

v2 vs baseline:
 - x arrives pre-transposed from the host (xT [E,S]) -> no on-chip x
   transposes (was 256 PE transposes + 64 scalar copies per core).
 - everything except PSUM and the final output is bf16: halves DMA +
   collective traffic, doubles DVE elementwise throughput.
 - softmax denominator accumulated on DVE (bf16 adds) instead of 160
   PE ones-matmuls; only the final partition-reduce runs on PE.
 - exp: off-diagonal score tiles are exp'd in [128,1024] pairs (halves
   ACTIVATE instruction overhead); diagonal tiles are exp'd only on the
   causal-valid column range.
 - causal mask: one [128,128] triangular bf16 multiply per diagonal tile
   (invalid prefix memset to 0 on gpsimd) instead of full [128,512] fp32
   multiplies.
 - softmax normalization (reduce/recip/broadcast/scale) for chunk qc is
   emitted during chunk qc+1 so the PE never stalls on the DVE chain.
 - RoPE rotate-half multiplies run on gpsimd, cos-mult + add on DVE.
"""

import sys
import time

sys.path.insert(0, "/opt/trn_rl_repo")

import numpy as np

import concourse.bass as bass
import concourse.mybir as mybir
import concourse.tile as tile
from concourse import bacc
from concourse.masks import make_identity

F32 = mybir.dt.float32
BF16 = mybir.dt.bfloat16
P = 128
HD = 128            # head dim
NHL = 4             # query heads per core
E = 2048            # hidden
DQ = NHL * HD       # 512, local q-projection width / o-slice width
SCALE = 1.0 / np.sqrt(np.float32(HD))
REPLICA_GROUPS = [[0, 1, 2, 3], [4, 5, 6, 7]]
NO_COLLECTIVE = False  # replace AllGather with a local DMA (timeline-sim only)
AG_HALVES = 1          # AllGathers per head (1, 2, or 4; must divide NQC)
DEN_DVE = True         # softmax denominator via DVE bf16 adds (else PE matmuls)
ROPE_GPSIMD = False    # rotate-half multiplies on gpsimd (else DVE)
DEN_RED = "pe"         # denominator partition-reduce: "pe" | "gpsimd"
LAG = 6                # pv(kt-LAG) emitted after scores(kt): hides exp+mask


def build_program(S=2048, reps=1, n_cores=8):
    """Build the per-core SPMD Bass program. Returns compiled nc."""
    ST = S // P          # 128-row tiles along sequence
    NQC = S // 512       # 512-wide chunks along sequence
    ET = E // P          # 16 tiles along hidden

    nc = bacc.Bacc("TRN2", target_bir_lowering=False, debug=False,
                   num_devices=n_cores)

    xT_in = nc.declare_dram_parameter("xT", [E, S], BF16, isOutput=False)
    wqT_in = nc.declare_dram_parameter("wqT", [E, DQ], BF16, isOutput=False)
    wkT_in = nc.declare_dram_parameter("wkT", [E, HD], BF16, isOutput=False)
    wvT_in = nc.declare_dram_parameter("wvT", [E, HD], BF16, isOutput=False)
    woT_in = nc.declare_dram_parameter("woT", [E, DQ], BF16, isOutput=False)
    cosT_in = nc.declare_dram_parameter("cosT", [HD, S], BF16, isOutput=False)
    sinT_in = nc.declare_dram_parameter("sinT", [HD, S], BF16, isOutput=False)
    out_d = nc.declare_dram_parameter("out", [DQ, S], F32, isOutput=True)

    with tile.TileContext(nc) as tc:
        with nc.allow_low_precision(reason="bf16 operands; tolerance 2e-2"):
            _emit(tc, nc, S, ST, NQC, ET, reps,
                  xT_in, wqT_in, wkT_in, wvT_in, woT_in, cosT_in, sinT_in,
                  out_d)

    nc.compile()
    return nc


def _emit(tc, nc, S, ST, NQC, ET, reps,
          xT_in, wqT_in, wkT_in, wvT_in, woT_in, cosT_in, sinT_in, out_d):
    from contextlib import ExitStack

    ctx = ExitStack()
    with ctx:
        const = ctx.enter_context(tc.tile_pool(name="const", bufs=1))
        qkv = ctx.enter_context(tc.tile_pool(name="qkv", bufs=2))
        dram = ctx.enter_context(tc.tile_pool(name="dram", bufs=2, space="DRAM"))

        # ---- constants ----
        ident = const.tile([P, P], BF16)
        make_identity(nc, ident[:])
        # triangular causal mask for the 128x128 diagonal blocks:
        # tri[k, j] = 1 if j >= k else 0
        tri = const.tile([P, P], BF16)
        nc.gpsimd.memset(tri[:], 1.0)
        nc.gpsimd.affine_select(
            out=tri[:], in_=tri[:],
            compare_op=mybir.AluOpType.is_ge,
            fill=0.0, base=0, pattern=[[1, P]],
            channel_multiplier=-1,
        )
        zeros_t = const.tile([P, 512], BF16)
        nc.gpsimd.memset(zeros_t[:], 0.0)
        ones_stage = const.tile([P, P], BF16)
        nc.gpsimd.memset(ones_stage[:], 1.0)
        ones_red = const.tile([P, 1], BF16)
        nc.vector.tensor_copy(ones_red[:], ones_stage[:, 0:1])
        ones_col = const.tile([1, P], BF16)
        nc.vector.tensor_copy(ones_col[:], ones_stage[0:1, :])


        pools = {
            "xn": ctx.enter_context(tc.tile_pool(name="xn", bufs=2)),
            "trig": ctx.enter_context(tc.tile_pool(name="trig", bufs=1)),
            "rope": ctx.enter_context(tc.tile_pool(name="rope", bufs=3)),
            "vt": ctx.enter_context(tc.tile_pool(name="vt", bufs=2)),
            "ex": ctx.enter_context(tc.tile_pool(name="ex", bufs=5)),
            "dn": ctx.enter_context(tc.tile_pool(name="dn", bufs=2)),
            "bc": ctx.enter_context(tc.tile_pool(name="bc", bufs=2)),
            "oh": ctx.enter_context(tc.tile_pool(name="oh", bufs=2)),
            "af": ctx.enter_context(tc.tile_pool(name="af", bufs=2)),
            "of": ctx.enter_context(tc.tile_pool(name="of", bufs=2)),
            # PSUM budget (8 banks): pj 2 + pt 1 + sc (2x[P,1024]) 4 + pv 1.
            # o_proj po tiles share the sc ring (doses run between chunks).
            "pj_ps": ctx.enter_context(tc.tile_pool(name="pj_ps", bufs=2, space="PSUM")),
            "pt_ps": ctx.enter_context(tc.tile_pool(name="pt_ps", bufs=1, space="PSUM")),
            "sc_ps": ctx.enter_context(tc.tile_pool(name="sc_ps", bufs=2, space="PSUM")),
            "pv_ps": ctx.enter_context(tc.tile_pool(name="pv_ps", bufs=1, space="PSUM")),
        }
        pools["qkv"] = qkv
        pools["dram"] = dram
        for rep in range(reps):
            _emit_rep(tc, nc, S, ST, NQC, ET, rep, pools,
                      xT_in, wqT_in, wkT_in, wvT_in, woT_in, cosT_in, sinT_in,
                      out_d, ident, tri, ones_red, ones_col, zeros_t)


def _emit_rep(tc, nc, S, ST, NQC, ET, rep, pools,
              xT_in, wqT_in, wkT_in, wvT_in, woT_in, cosT_in, sinT_in,
              out_d, ident, tri, ones_red, ones_col, zeros_t):
    import concourse.bass_isa as bass_isa

    NHALF = min(AG_HALVES, NQC)
    cph = NQC // NHALF
    out_r = out_d.rearrange("(ot p) s -> p ot s", p=P)

    # per-rep Q/K/V buffers (double-buffered pool) so rep r+1's projections
    # can overlap rep r's attention tail
    QT_sb = pools["qkv"].tile([P, NHL, S], BF16, name="QT", tag="QT")
    KT_sb = pools["qkv"].tile([P, S], BF16, name="KT", tag="KT")
    V_sb = pools["qkv"].tile([P, ST, HD], BF16, name="V", tag="V")
    SH = S // NHALF
    agin = [[pools["dram"].tile([P, SH], BF16, name=f"agin{h}_{hf}",
                                tag=f"agin{h}_{hf}")
             for hf in range(NHALF)] for h in range(NHL)]
    agout = [[pools["dram"].tile([4 * P, SH], BF16, name=f"agout{h}_{hf}",
                                 tag=f"agout{h}_{hf}")
              for hf in range(NHALF)] for h in range(NHL)]

    if True:
        xn_pool = pools["xn"]
        trig_pool = pools["trig"]
        rope_pool = pools["rope"]
        vt_pool = pools["vt"]
        ex_pool = pools["ex"]
        dn_pool = pools["dn"]
        bc_pool = pools["bc"]
        oh_pool = pools["oh"]
        af_pool = pools["af"]
        of_pool = pools["of"]
        pj_ps = pools["pj_ps"]
        pt_ps = pools["pt_ps"]
        sc_ps = pools["sc_ps"]
        pv_ps = pools["pv_ps"]

        cosT_sb = trig_pool.tile([P, S], BF16)
        sinT_sb = trig_pool.tile([P, S], BF16)
        wqT_sb = trig_pool.tile([P, ET, DQ], BF16)
        wkT_sb = trig_pool.tile([P, ET, HD], BF16)
        wvT_sb = trig_pool.tile([P, ET, HD], BF16)
        woT_sb = trig_pool.tile([P, ET, DQ], BF16)

        x_r = xT_in.rearrange("(et p) s -> p et s", p=P)
        wq_r = wqT_in.rearrange("(et p) d -> p et d", p=P)
        wk_r = wkT_in.rearrange("(et p) d -> p et d", p=P)
        wv_r = wvT_in.rearrange("(et p) d -> p et d", p=P)
        wo_r = woT_in.rearrange("(et p) d -> p et d", p=P)

        # ---------------- per-chunk emitters ----------------

        def emit_proj(qc):
            s0 = qc * 512
            xc = xn_pool.tile([P, ET, 512], BF16, name="xc", tag="xc")
            for q4 in range(4):
                nc.sync.dma_start(xc[:, q4 * 4:(q4 + 1) * 4, :],
                                  x_r[:, q4 * 4:(q4 + 1) * 4, s0:s0 + 512])
                if qc == 0:
                    # interleave weight-slice DMAs with the first chunk's
                    # x loads so the first matmuls aren't starved
                    et4 = slice(q4 * 4, (q4 + 1) * 4)
                    nc.sync.dma_start(wqT_sb[:, et4, :], wq_r[:, et4, :])
                    nc.sync.dma_start(wkT_sb[:, et4, :], wk_r[:, et4, :])
                    nc.sync.dma_start(wvT_sb[:, et4, :], wv_r[:, et4, :])
            # cos/sin stream in per-chunk; wo isn't needed until the first
            # o_proj dose, so its load is deferred off the critical start
            nc.sync.dma_start(cosT_sb[:, s0:s0 + 512], cosT_in[:, s0:s0 + 512])
            nc.sync.dma_start(sinT_sb[:, s0:s0 + 512], sinT_in[:, s0:s0 + 512])
            if qc == min(1, NQC - 1):
                for q4 in range(4):
                    et4 = slice(q4 * 4, (q4 + 1) * 4)
                    nc.sync.dma_start(woT_sb[:, et4, :], wo_r[:, et4, :])

            cos_c = cosT_sb[:, s0:s0 + 512]
            sin_c = sinT_sb[:, s0:s0 + 512]
            for d6 in range(6):
                pp = pj_ps.tile([P, 512], F32, name="pp", tag="pp")
                for et in range(ET):
                    if d6 < 4:
                        lhsT = wqT_sb[:, et, d6 * HD:(d6 + 1) * HD]
                    elif d6 == 4:
                        lhsT = wkT_sb[:, et, :]
                    else:
                        lhsT = wvT_sb[:, et, :]
                    nc.tensor.matmul(pp[:], lhsT, xc[:, et, :],
                                     start=(et == 0), stop=(et == ET - 1))
                if d6 < 5:
                    dst = (QT_sb[:, d6, s0:s0 + 512] if d6 < 4
                           else KT_sb[:, s0:s0 + 512])
                    ppb = rope_pool.tile([P, 512], BF16, name="ppb", tag="ppb")
                    ppr = rope_pool.tile([P, 512], BF16, name="ppr", tag="ppr")
                    nc.scalar.copy(ppb[:], pp[:])
                    # rotate-half folded into the PSUM->SBUF evacuation
                    nc.scalar.copy(ppr[0:64, :], pp[64:128, :])
                    nc.scalar.copy(ppr[64:128, :], pp[0:64, :])
                    t1 = rope_pool.tile([P, 512], BF16, name="t1", tag="t1")
                    t2 = rope_pool.tile([P, 512], BF16, name="t2", tag="t2")
                    nc.vector.tensor_tensor(t1[:], ppb[:], cos_c,
                                            mybir.AluOpType.mult)
                    # sinT arrives with rows 0:64 pre-negated (host side)
                    eng = nc.gpsimd if ROPE_GPSIMD else nc.vector
                    eng.tensor_tensor(t2[:], ppr[:], sin_c,
                                      mybir.AluOpType.mult)
                    nc.vector.tensor_tensor(dst, t1[:], t2[:],
                                            mybir.AluOpType.add)
                else:
                    vts = vt_pool.tile([P, 512], BF16, name="vts", tag="vts")
                    nc.scalar.copy(vts[:], pp[:])
                    for st4 in range(4):
                        pv_t = pt_ps.tile([P, P], BF16, name="pvt",
                                          tag="ptile")
                        nc.tensor.transpose(pv_t[:],
                                            vts[:, st4 * P:(st4 + 1) * P],
                                            ident[:])
                        nc.scalar.copy(V_sb[:, qc * 4 + st4, :], pv_t[:])

        pending = [None]

        def norm_stage1(pv, denacc, h, qc):
            if DEN_RED == "pe":
                # partition-reduce both denacc halves on PE (accumulating),
                # using half a sc-ring tile
                pdred = sc_ps.tile([P, 1024], F32, name="pdred", tag="ps2")
                nc.tensor.matmul(pdred[0:1, 0:512], ones_red[:],
                                 denacc[:, 0:512], start=True, stop=False)
                nc.tensor.matmul(pdred[0:1, 0:512], ones_red[:],
                                 denacc[:, 512:1024], start=False, stop=True)
                rec = bc_pool.tile([1, 512], BF16, name="rec", tag="rec")
                nc.vector.reciprocal(rec[:], pdred[0:1, 0:512])
                return (pv, rec, h, qc)
            else:
                dfold = dn_pool.tile([P, 512], BF16, name="dfold",
                                     tag="dfold")
                nc.vector.tensor_add(dfold[:], denacc[:, 0:512],
                                     denacc[:, 512:1024])
                denred = dn_pool.tile([P, 512], BF16, name="denred",
                                      tag="denred")
                nc.gpsimd.partition_all_reduce(
                    denred[:], dfold[:], channels=P,
                    reduce_op=bass_isa.ReduceOp.add)
                bcr = bc_pool.tile([P, 512], BF16, name="bcr", tag="bcr")
                nc.vector.reciprocal(bcr[:], denred[:])
                return (pv, bcr, h, qc)

        def norm_stage2(st):
            pv, rec, h, qc = st
            if DEN_RED == "pe":
                pbc = sc_ps.tile([P, 1024], F32, name="pbc", tag="ps2")
                nc.tensor.matmul(pbc[:, 0:512], ones_col[:], rec[:],
                                 start=True, stop=True)
                bcr = bc_pool.tile([P, 512], BF16, name="bcr", tag="bcr")
                nc.scalar.copy(bcr[:], pbc[:, 0:512])
            else:
                bcr = rec
            outH = oh_pool.tile([P, 512], BF16, name="outH", tag="outH")
            nc.vector.tensor_tensor(outH[:], pv[:], bcr[:],
                                    mybir.AluOpType.mult)
            hf = qc // cph
            qh0 = (qc - hf * cph) * 512
            nc.sync.dma_start(agin[h][hf][:, qh0:qh0 + 512], outH[:])

        def flush_pending():
            if pending[0] is not None:
                norm_stage2(norm_stage1(*pending[0]))
                pending[0] = None

        def emit_attn(h, qc):
            q0 = qc * 512
            nkt = 4 * qc + 4
            noff = 4 * qc            # fully-valid (off-diagonal) tiles
            qT = QT_sb[:, h, q0:q0 + 512]
            pv = pv_ps.tile([P, 512], F32, name="pv", tag="pv")
            denacc = dn_pool.tile([P, 1024], BF16, name="denacc",
                                  tag="denacc")
            exs = [None] * nkt
            den_started = [False, False]   # per half of denacc
            stage_state = [None]

            def emit_pv(j, last):
                t = j - noff
                c0 = P * t if t > 0 else 0   # diag tiles: ex is 0 below c0
                nc.tensor.matmul(pv[:, c0:512], V_sb[:, j, :],
                                 exs[j][:, c0:512],
                                 start=(j == 0), stop=last)

            ps2 = None
            ex2 = None
            for kt in range(nkt):
                half = kt % 2
                off = half * 512
                if half == 0:
                    ps2 = sc_ps.tile([P, 1024], F32, name="ps2", tag="ps2")
                    ex2 = ex_pool.tile([P, 1024], BF16, name="ex2",
                                       tag="ex2")
                t = kt - noff        # >= 0 -> diagonal tile
                if t >= 0:
                    c0 = P * t
                    nc.tensor.matmul(ps2[:, off + c0:off + 512],
                                     KT_sb[:, kt * P:(kt + 1) * P],
                                     qT[:, c0:512],
                                     start=True, stop=True)
                    # per-tile exp on the causal-valid range only
                    nc.scalar.activation(
                        ex2[:, off + c0:off + 512],
                        ps2[:, off + c0:off + 512],
                        mybir.ActivationFunctionType.Exp,
                        scale=float(SCALE))
                    if c0 > 0:
                        nc.vector.tensor_copy(ex2[:, off:off + c0],
                                              zeros_t[:, 0:c0])
                    nc.vector.tensor_tensor(
                        ex2[:, off + c0:off + c0 + P],
                        ex2[:, off + c0:off + c0 + P],
                        tri[:], mybir.AluOpType.mult)
                else:
                    nc.tensor.matmul(ps2[:, off:off + 512],
                                     KT_sb[:, kt * P:(kt + 1) * P],
                                     qT, start=True, stop=True)
                    if half == 1:
                        # paired exp over both halves
                        nc.scalar.activation(
                            ex2[:, :], ps2[:, :],
                            mybir.ActivationFunctionType.Exp,
                            scale=float(SCALE))
                exs[kt] = ex2[:, off:off + 512]
                # denominator accumulation (DVE bf16), pair-granular where
                # possible. Emitted only after the pair's exp exists so the
                # reads bind to the right writer of the recycled ex2 buffer.
                if t >= 0:
                    dhalf = kt % 2
                    dslice = denacc[:, dhalf * 512:(dhalf + 1) * 512]
                    if not den_started[dhalf]:
                        nc.vector.tensor_copy(dslice, exs[kt])
                        den_started[dhalf] = True
                    else:
                        nc.vector.tensor_add(dslice, dslice, exs[kt])
                elif half == 1:
                    if not den_started[0]:
                        # first off-diag pair: one [P,1024] copy seeds both
                        nc.vector.tensor_copy(denacc[:], ex2[:, :])
                        den_started = [True, True]
                    else:
                        nc.vector.tensor_add(denacc[:], denacc[:], ex2[:, :])
                if kt == 1 and pending[0] is not None:
                    stage_state[0] = norm_stage1(*pending[0])
                    pending[0] = None
                if kt == 3 and stage_state[0] is not None:
                    norm_stage2(stage_state[0])
                    stage_state[0] = None
                if kt >= LAG:
                    emit_pv(kt - LAG, last=False)
            for j in range(max(0, nkt - LAG), nkt):
                emit_pv(j, last=(j == nkt - 1))
            if stage_state[0] is not None:
                norm_stage2(stage_state[0])
                stage_state[0] = None

            # softmax normalization is deferred: stages are emitted during
            # the NEXT chunk so the PE pipeline never waits on the chain
            pending[0] = (pv, denacc, h, qc)

        def emit_ag(hf):
            for h in range(NHL):
                if NO_COLLECTIVE:
                    for mt in range(4):
                        nc.sync.dma_start(
                            agout[h][hf][mt * P:(mt + 1) * P, :],
                            agin[h][hf][:])
                else:
                    nc.gpsimd.collective_compute(
                        "AllGather", mybir.AluOpType.bypass,
                        replica_groups=REPLICA_GROUPS,
                        ins=[agin[h][hf].opt()],
                        outs=[agout[h][hf].opt()])

        def prefetch_af(hf, sch):
            afs = []
            for h in range(NHL):
                af = af_pool.tile([P, 4, 512], BF16, name=f"af{h}",
                                  tag=f"af{h}")
                ag_r = agout[h][hf].rearrange("(mt p) s -> p mt s", p=P)
                nc.sync.dma_start(
                    af[:], ag_r[:, :, sch * 512:(sch + 1) * 512])
                afs.append(af)
            return afs

        def emit_oproj_ot(hf, sch, afs, ot):
            # one o_proj output tile: accumulate over (mt, head) in PSUM --
            # all four heads' AllGathers for this half are done
            sc = hf * cph + sch
            s0 = sc * 512
            po = sc_ps.tile([P, 1024], F32, name="po", tag="ps2")
            first = True
            for mt in range(4):
                for h in range(NHL):
                    nc.tensor.matmul(
                        po[:, 0:512],
                        woT_sb[:, 4 * mt + h, ot * P:(ot + 1) * P],
                        afs[h][:, mt, :],
                        start=first, stop=(mt == 3 and h == NHL - 1))
                    first = False
            of = of_pool.tile([P, 512], F32, name="of", tag="of")
            nc.scalar.copy(of[:], po[:, 0:512])
            nc.sync.dma_start(out_r[:, ot, s0:s0 + 512], of[:])

        # ---------------- qc-major schedule ----------------
        from collections import deque
        dose_q = deque()     # (hf, sch, afs, ot) pending o_proj tiles

        def enqueue_doses(hf):
            for sch in range(cph):
                afs = prefetch_af(hf, sch)
                for ot in range(4):
                    dose_q.append((hf, sch, afs, ot))

        for qc in range(NQC):
            emit_proj(qc)
            for h in range(NHL):
                emit_attn(h, qc)
                if dose_q:
                    emit_oproj_ot(*dose_q.popleft())
            if (qc + 1) % cph == 0:
                hf = qc // cph
                flush_pending()
                emit_ag(hf)
                enqueue_doses(hf)
        flush_pending()
        while dose_q:
            emit_oproj_ot(*dose_q.popleft())


# ======================= host side =======================

_CACHE = {}


def _get_program(S=2048, reps=1):
    key = (S, reps, AG_HALVES, NO_COLLECTIVE, DEN_DVE, ROPE_GPSIMD)
    if key not in _CACHE:
        _CACHE[key] = build_program(S=S, reps=reps)
    return _CACHE[key]


def make_in_maps(x, cos, sin, wq, wk, wv, wo):
    bf = mybir.dt.np(BF16)
    in_maps = []
    cosT = np.ascontiguousarray(cos.T.astype(np.float32)).astype(bf)
    sinT = sin.T.astype(np.float32).copy()
    sinT[:HD // 2, :] *= -1.0   # fold rotate_half sign into the table
    sinT = np.ascontiguousarray(sinT).astype(bf)
    xTs = [np.ascontiguousarray(np.asarray(x[b]).T.astype(np.float32)).astype(bf)
           for b in range(x.shape[0])]
    wqTs = [np.ascontiguousarray(wq[g * DQ:(g + 1) * DQ, :].T.astype(np.float32)).astype(bf)
            for g in range(4)]
    wkTs = [np.ascontiguousarray(wk[g * HD:(g + 1) * HD, :].T.astype(np.float32)).astype(bf)
            for g in range(4)]
    wvTs = [np.ascontiguousarray(wv[g * HD:(g + 1) * HD, :].T.astype(np.float32)).astype(bf)
            for g in range(4)]
    woTs = [np.ascontiguousarray(wo[:, :].T[:, g * DQ:(g + 1) * DQ].astype(np.float32)).astype(bf)
            for g in range(4)]
    for c in range(8):
        b, g = c // 4, c % 4
        in_maps.append({
            "xT": xTs[b],
            "wqT": wqTs[g],
            "wkT": wkTs[g],
            "wvT": wvTs[g],
            "woT": woTs[g],
            "cosT": cosT,
            "sinT": sinT,
        })
    return in_maps


def assemble_output(results, B, S):
    out = np.empty((B, S, E), np.float32)
    for c in range(8):
        b, g = c // 4, c % 4
        out[b][:, g * DQ:(g + 1) * DQ] = results[c]["out"].T
    return out


# ---- inline SPMD runner (PJRT/axon), device-resident inputs ----

class SpmdRunner:
    def __init__(self, nc, n_cores):
        import jax
        from jax.sharding import Mesh, PartitionSpec
        from jax.experimental.shard_map import shard_map
        from concourse import bass2jax
        from concourse.bass2jax import _bass_exec_p, install_neuronx_cc_hook

        install_neuronx_cc_hook()
        self.jax = jax
        self.nc = nc
        self.n_cores = n_cores
        partition_name = (nc.partition_id_tensor.name
                          if nc.partition_id_tensor else None)
        in_names, out_names, out_avals = [], [], []
        zero_outs = []
        for alloc in nc.m.functions[0].allocations:
            if not isinstance(alloc, mybir.MemoryLocationSet):
                continue
            name = alloc.memorylocations[0].name
            if alloc.kind == "ExternalInput":
                if name != partition_name:
                    in_names.append(name)
            elif alloc.kind == "ExternalOutput":
                out_names.append(name)
                shape = tuple(alloc.tensor_shape)
                dtype = mybir.dt.np(alloc.dtype)
                out_avals.append(jax.core.ShapedArray(shape, dtype))
                zero_outs.append(np.zeros(shape, dtype))
        self.in_names, self.out_names = in_names, out_names
        self.out_avals, self.zero_outs = out_avals, zero_outs
        self.n_params = len(in_names)

        all_in = list(in_names) + list(out_names)
        if partition_name is not None:
            all_in.append(partition_name)

        def _body(*args):
            operands = list(args)
            if partition_name is not None:
                operands.append(bass2jax.partition_id_tensor())
            outs = _bass_exec_p.bind(
                *operands, out_avals=tuple(out_avals),
                in_names=tuple(all_in), out_names=tuple(out_names),
                lowering_input_output_aliases=(),
                sim_require_finite=True, sim_require_nnan=True, nc=nc)
            return tuple(outs)

        devices = jax.devices()[:n_cores]
        self.mesh = Mesh(np.asarray(devices), ("core",))
        n_outs = len(out_names)
        in_specs = (PartitionSpec("core"),) * (self.n_params + n_outs)
        out_specs = (PartitionSpec("core"),) * n_outs
        self.fn = jax.jit(
            shard_map(_body, mesh=self.mesh, in_specs=in_specs,
                      out_specs=out_specs, check_rep=False),
            keep_unused=True)
        self.dev_args = None

    def stage_inputs(self, in_maps):
        import jax
        from jax.sharding import PartitionSpec
        per_core = [[np.asarray(m[n]) for n in self.in_names] for m in in_maps]
        concat_in = [
            np.concatenate([per_core[c][i] for c in range(self.n_cores)], axis=0)
            for i in range(self.n_params)]
        concat_zeros = [
            np.zeros((self.n_cores * z.shape[0], *z.shape[1:]), z.dtype)
            for z in self.zero_outs]
        sharding = jax.sharding.NamedSharding(self.mesh, PartitionSpec("core"))
        self.dev_args = [jax.device_put(a, sharding)
                         for a in (*concat_in, *concat_zeros)]
        for a in self.dev_args:
            a.block_until_ready()

    def run(self):
        out_arrs = [np.asarray(o) for o in self.fn(*self.dev_args)]
        return [
            {n: out_arrs[i].reshape(self.n_cores, *self.out_avals[i].shape)[c]
             for i, n in enumerate(self.out_names)}
            for c in range(self.n_cores)]

    def time_exec(self, iters=30, warmup=3):
        import jax
        for _ in range(warmup):
            res = self.fn(*self.dev_args)
        jax.block_until_ready(res)
        t0 = time.perf_counter()
        for _ in range(iters):
            res = self.fn(*self.dev_args)
        jax.block_until_ready(res)
        t1 = time.perf_counter()
        return (t1 - t0) / iters * 1e9


_RUNNER_CACHE = {}


def get_runner(S=2048, reps=1):
    key = (S, reps, AG_HALVES, NO_COLLECTIVE, DEN_DVE, ROPE_GPSIMD)
    if key not in _RUNNER_CACHE:
        nc = _get_program(S=S, reps=reps)
        _RUNNER_CACHE[key] = SpmdRunner(nc, 8)
    return _RUNNER_CACHE[key]


def kernel(x, cos, sin, wq, wk, wv, wo):
    B, S, _ = x.shape
    runner = get_runner(S=S, reps=1)
    runner.stage_inputs(make_in_maps(x, cos, sin, wq, wk, wv, wo))
    results = runner.run()
    return assemble_output(results, B, S)


if __name__ == "__main__":
    # tiny self-test against a local numpy reference
    S = int(sys.argv[1]) if len(sys.argv) > 1 else 512
    rng = np.random.default_rng(0)
    B, H, HKV = 2, 16, 4
    x = rng.standard_normal((B, S, E), dtype=np.float32)
    cos = rng.random((S, HD), dtype=np.float32)
    sin = rng.random((S, HD), dtype=np.float32)
    sc = 0.02
    wq = (rng.standard_normal((H * HD, E), dtype=np.float32) * sc)
    wk = (rng.standard_normal((HKV * HD, E), dtype=np.float32) * sc)
    wv = (rng.standard_normal((HKV * HD, E), dtype=np.float32) * sc)
    wo = (rng.standard_normal((E, H * HD), dtype=np.float32) * sc)

    def ref(x, cos, sin, wq, wk, wv, wo):
        x64 = x.astype(np.float64)
        q = (x64 @ wq.T.astype(np.float64)).reshape(B, S, H, HD)
        k = (x64 @ wk.T.astype(np.float64)).reshape(B, S, HKV, HD)
        v = (x64 @ wv.T.astype(np.float64)).reshape(B, S, HKV, HD)

        def rot(t):
            return np.concatenate([-t[..., HD // 2:], t[..., :HD // 2]], -1)

        c = cos[:, None, :].astype(np.float64)
        s = sin[:, None, :].astype(np.float64)
        q = q * c + rot(q) * s
        k = k * c + rot(k) * s
        k = np.repeat(k, H // HKV, axis=2).transpose(0, 2, 1, 3)
        v = np.repeat(v, H // HKV, axis=2).transpose(0, 2, 1, 3)
        q = q.transpose(0, 2, 1, 3)
        scores = np.einsum("bhqd,bhkd->bhqk", q, k) / np.sqrt(HD)
        mask = np.tril(np.ones((S, S), bool))
        scores = np.where(mask, scores, -np.inf)
        scores -= scores.max(-1, keepdims=True)
        p = np.exp(scores)
        p /= p.sum(-1, keepdims=True)
        o = np.einsum("bhqk,bhkd->bhqd", p, v)
        o = o.transpose(0, 2, 1, 3).reshape(B, S, H * HD)
        return o @ wo.T.astype(np.float64)

    want = ref(x, cos, sin, wq, wk, wv, wo)
    got = kernel(x, cos, sin, wq, wk, wv, wo)
    err = np.abs(got - want).max() / np.abs(want).max()
    print(f"S={S}: rel err (absmax-relative) = {err:.3e}")
